# revision 47
# baseline (speedup 1.0000x reference)
"""Self-contained Trainium2 Bass kernel for causal multi-head attention.

Problem: B=2, S=2048, D=1024, H=16 heads (dk=64), fp32, causal + padding mask.
Sharding across 8 NeuronCores: core c -> batch c//4, head-group c%4 (4 heads).

Device-side design:
  - x, Wq, Wk, Wv cast to bf16 on host: input DMA halves (12MB -> 6MB) and
    every matmul streams 1 col/cycle (fp32r at K=64 ran at 2 cyc/col).
  - qT/kT stored transposed [dk, S] in bf16; scores computed transposed
    S_T[k, q] so no transposes are needed anywhere; scores/exp run one key
    block ahead of PV (software pipeline depth 2).
  - No max-subtraction in softmax (scores are O(+-10); exp cannot overflow).
  - vaug per key block, per head pair (192 cols):
      [v_even(64) | den_e | junk(31) | den_o | junk(31) | v_odd(64)]
    Even PV stationary = cols 0:65 -> psum rows 0:64 ctx_e, row 64 den_e.
    Odd PV stationary = cols 64:192 -> psum row 32 den_o (quadrant-aligned
    for the DVE copy), rows 64:128 ctx_o. This lands the odd head's context
    at partitions 64:128 for free, so the output projection runs with K=128
    fully utilized (2 heads per accumulation step).
  - Padding: v rows and den cols zeroed for padded keys, so garbage exp
    values at padded keys multiply zeros everywhere.
  - Causal: tri (-8e9 lower triangle, bf16) accumulated onto the diagonal
    score blocks by the PE itself (eye-stationary matmul) — keeps the DVE
    out of the scores critical path.
  - Q/K psum->SBUF copies and the crawA context copy run on the Act engine
    (Identity-with-bias / Copy share Exp's act table, so no table reloads);
    V scaling stays on DVE.
  - The attention inner loop is exp-paced (Act engine); output-projection
    matmuls are sliced into single-matmul quanta and dripped one-per-key-
    block into the next chunk's attention loop to absorb the PE idle.
  - 1/denominator broadcast to the pair's 128 partitions via two K=1 bf16
    PE matmuls (the broadcasts ride idle PE slots; no DRAM bounce), then
    multiplied into fp32r ctx pair tiles.
  - Chunks processed in order 1,2,3,0 so the tail chunk is the smallest;
    projections for chunks 2,3 ride inside qc=1's pairs.
Fully-masked rows (all keys up to q padded) produce garbage on device and
are overwritten on host with the uniform-attention reference value.
"""

import numpy as np
from contextlib import ExitStack

import ml_dtypes

import concourse.bass as bass
import concourse.bacc as bacc
import concourse.tile as tile
import concourse.mybir as mybir
from concourse.bass import ds, ts

F32 = mybir.dt.float32
FR = mybir.dt.float32r
BF = mybir.dt.bfloat16
AF = mybir.ActivationFunctionType

P = 128
S = 2048
D = 1024
HL = 4          # heads per core
DK = 64
KT = D // P     # 8 k-tiles over the model dim
ST = S // P     # 16 seq tiles
NQC = 4         # 512-wide query chunks
NEG = -8.0e9    # pre-scale mask value; *0.125 = -1e9 -> exp underflows to 0
N_CORES = 8
N_HEAD = 16

PW = 192                     # cols per head pair in vaug (aligned den rows)
QC_ORDER = (1, 2, 3, 0)      # tail chunk is the smallest
CW = 384                     # compacted keys per original 512-token chunk
SK = 4 * CW                  # compacted key axis (1536)
SKT = SK // P                # 12 compacted key tiles


def build_program(num_devices=N_CORES):
    nc = bacc.Bacc(
        "TRN2",
        target_bir_lowering=False,
        debug=False,
        enable_asserts=True,
        num_devices=num_devices,
    )
    ins = {
        "xt": nc.dram_tensor("xt", [D, S], BF, kind="ExternalInput").ap(),
        "wq": nc.dram_tensor("wq", [D, 2 * P], BF, kind="ExternalInput").ap(),
        "wk": nc.dram_tensor("wk", [D, 2 * P], BF, kind="ExternalInput").ap(),
        "wv": nc.dram_tensor("wv", [D, 2 * P], BF, kind="ExternalInput").ap(),
        "wo": nc.dram_tensor("wo", [2 * P, D], FR, kind="ExternalInput").ap(),
        "bq": nc.dram_tensor("bq", [P, 2], F32, kind="ExternalInput").ap(),
        "pad01": nc.dram_tensor("pad01", [P, SKT], F32, kind="ExternalInput").ap(),
        "eye": nc.dram_tensor("eye", [P, P], BF, kind="ExternalInput").ap(),
        "xtk": nc.dram_tensor("xtk", [D, SK], BF, kind="ExternalInput").ap(),
        "masks": nc.dram_tensor("masks", [P, 12 * 512], BF, kind="ExternalInput").ap(),
    }
    y = nc.dram_tensor("y", [S, D], F32, kind="ExternalOutput").ap()

    with tile.TileContext(nc) as tc:
        _body(tc, y, ins)

    nc.compile()
    return nc


def _body(tc, y, ins):
    nc = tc.nc

    with ExitStack() as ctx:
        const = ctx.enter_context(tc.tile_pool(name="const", bufs=1))
        pt_pool = ctx.enter_context(tc.tile_pool(name="pt", bufs=4))
        rrp = ctx.enter_context(tc.tile_pool(name="rr", bufs=2))
        ysb = ctx.enter_context(tc.tile_pool(name="ysb", bufs=2))
        psA = ctx.enter_context(tc.tile_pool(name="psA", bufs=2, space="PSUM"))
        psB = ctx.enter_context(tc.tile_pool(name="psB", bufs=2, space="PSUM"))
        psY = ctx.enter_context(tc.tile_pool(name="psY", bufs=2, space="PSUM"))

        # warmup operands first so the PE ramps while inputs stream
        ones_sb = const.tile([1, 512], BF)
        nc.vector.memset(ones_sb[:], 1.0)
        ones_bf = const.tile([1, DK], BF)
        nc.vector.memset(ones_bf[:], 1.0)

        # ---------------- input DMAs ----------------
        # all weights first (small, spread over the queues), then xt chunks
        xt_sb = const.tile([P, KT, S], BF)
        wq_sb = const.tile([P, KT, 2 * P], BF)
        wk_sb = const.tile([P, KT, 2 * P], BF)
        wv_sb = const.tile([P, KT, 2 * P], BF)
        xt_r = ins["xt"].rearrange("(k p) s -> k p s", p=P)
        w_rs = {n: ins[n].rearrange("(k p) n -> k p n", p=P) for n in ("wq", "wk", "wv")}

        bq_sb = const.tile([P, 2], F32)
        nc.sync.dma_start(bq_sb[:], ins["bq"])
        pad01_sb = const.tile([P, SKT], F32)
        nc.sync.dma_start(pad01_sb[:], ins["pad01"])
        eye_sb = const.tile([P, P], BF)
        nc.sync.dma_start(eye_sb[:], ins["eye"])
        masks_sb = const.tile([P, 12, 512], BF)
        nc.sync.dma_start(
            masks_sb[:], ins["masks"].rearrange("p (m q) -> p m q", q=512)
        )
        xtk_sb = const.tile([P, KT, SK], BF)
        xtk_r = ins["xtk"].rearrange("(k p) s -> k p s", p=P)

        # xt chunk 0 first — each tile gets its own queue with nothing ahead
        # of it (the PE warmup covers the weight-DMA latency); then weights,
        # then the compacted-key chunks interleaved with the later q chunks
        for k in range(KT):
            nc.sync.dma_start(xt_sb[:, k, 0:512], xt_r[k][:, 0:512])
        for k in range(KT):
            nc.sync.dma_start(wq_sb[:, k], w_rs["wq"][k])
            nc.sync.dma_start(wk_sb[:, k], w_rs["wk"][k])
        for k in range(KT):
            nc.sync.dma_start(wv_sb[:, k], w_rs["wv"][k])
            nc.sync.dma_start(xtk_sb[:, k, 0:512], xtk_r[k][:, 0:512])
        for k in range(KT):
            nc.sync.dma_start(xt_sb[:, k, 512:1024], xt_r[k][:, 512:1024])
            nc.sync.dma_start(xtk_sb[:, k, 512:1024], xtk_r[k][:, 512:1024])
        for k in range(KT):
            nc.sync.dma_start(xt_sb[:, k, 1024:2048], xt_r[k][:, 1024:2048])
            nc.sync.dma_start(xtk_sb[:, k, 1024:1536], xtk_r[k][:, 1024:1536])

        # wo rows for a head pair are contiguous 128 rows -> K=128 all real
        wo_sb = const.tile([P, 2, D], FR)
        wo_r = ins["wo"].rearrange("(hp p) n -> hp p n", p=P)
        for hp in range(2):
            nc.sync.dma_start(wo_sb[:, hp], wo_r[hp])

        qt_sb = const.tile([P, 2, S], BF)
        kt_sb = const.tile([P, 2, SK], BF)
        vaug_sb = const.tile([P, SKT, 2 * PW], BF)
        for m in range(2):
            nc.vector.memset(vaug_sb[:, :, m * PW + DK + 1 : m * PW + 2 * DK], 0.0)

        # normalized context pairs [even dims 0:64 | odd dims 64:128], fp32r
        ctx_sets = [
            [
                const.tile([P, 512], FR, name=f"ctxsb{st}_{hp}", tag=f"ctxsb{st}_{hp}")
                for hp in range(2)
            ]
            for st in range(2)
        ]

        # PE warmup while the input DMAs stream (HAM un-throttle needs
        # ~3.4us of sustained matmul activity; these are dep-free)
        warm_ps = psY.tile([P, 512], F32, name="warm", tag="yp")

        def warm(n):
            """Dep-free PE filler: keeps the HAM clock ramped across input-
            DMA catch-up waits (a throttled PE runs matmuls at 1.2GHz)."""
            for _ in range(n):
                nc.tensor.matmul(
                    warm_ps[:], ones_sb[:, 0:P], ones_sb[:], start=True, stop=True
                )

        warm(24)

        # ---------------- projections for one 512-token chunk ----------------
        def proj_q(n):
            ps = psA.tile([P, 1024], F32, name=f"ps_q{n}", tag="ps")
            for m in range(2):
                for k in range(KT):
                    nc.tensor.matmul(
                        ps[:, ts(m, 512)],
                        wq_sb[:, k, ts(m, P)],
                        xt_sb[:, k, ds(n * 512, 512)],
                        start=(k == 0),
                        stop=(k == KT - 1),
                    )
            for m in range(2):
                nc.scalar.activation(
                    qt_sb[:, m, ds(n * 512, 512)], ps[:, ts(m, 512)],
                    AF.Identity, bias=bq_sb[:, m : m + 1],
                )

        def proj_kv(n):
            """K and V projections on one 512-col chunk of the compacted
            key axis."""
            ps = psA.tile([P, 1024], F32, name=f"ps_k{n}", tag="ps")
            for m in range(2):
                for k in range(KT):
                    nc.tensor.matmul(
                        ps[:, ts(m, 512)],
                        wk_sb[:, k, ts(m, P)],
                        xtk_sb[:, k, ds(n * 512, 512)],
                        start=(k == 0),
                        stop=(k == KT - 1),
                    )
            for m in range(2):
                nc.scalar.activation(
                    kt_sb[:, m, ds(n * 512, 512)], ps[:, ts(m, 512)], AF.Copy
                )
            ps = psA.tile([P, 1024], F32, name=f"ps_v{n}", tag="ps")
            for si in range(4):
                s = n * 4 + si
                for k in range(KT):
                    nc.tensor.matmul(
                        ps[:, ts(si, 256)],
                        xtk_sb[:, k, ts(s, P)],
                        wv_sb[:, k, :],
                        start=(k == 0),
                        stop=(k == KT - 1),
                    )
            for si in range(4):
                s = n * 4 + si
                for h in range(HL):
                    m, odd = divmod(h, 2)
                    dst = m * PW + (2 * DK if odd else 0)
                    nc.vector.tensor_scalar_mul(
                        vaug_sb[:, s, ds(dst, DK)],
                        ps[:, ds(si * 256 + h * DK, DK)],
                        pad01_sb[:, s : s + 1],
                    )
                den_ap = vaug_sb[:, s, 0 : 2 * PW].rearrange(
                    "p (g c) -> p g c", c=32
                )
                for m in range(2):
                    nc.vector.tensor_copy(
                        den_ap[:, 6 * m + 2 : 6 * m + 4, 0:1],
                        pad01_sb[:, s : s + 1].to_broadcast([P, 2, 1]),
                    )

        # ---------------- attention for one 512-query chunk ----------------
        y_r = y.rearrange("(t p) n -> t p n", p=P)

        def scores_pair(qc, m, fill=()):
            """QK^T, exp, PV for head pair (2m, 2m+1) over the compacted key
            blocks: 3 exact-causal blocks per past chunk (no mask) plus this
            chunk's own 3 ragged-diagonal blocks (host-built additive masks
            applied by the PE). `fill` items are dripped one per key block
            from fill_start to soak up the exp-paced PE idle."""
            fill = list(fill)
            nkb = 3 * (qc + 1)
            fill_start = 2 if nkb <= 4 else 4
            vb = m * PW
            pvs = [
                psB.tile([P, 512], F32, name=f"ctx{qc}_{m}_{i}", tag="ctx")
                for i in range(2)
            ]

            def emit_pv(kb, pt, qoff):
                nc.tensor.matmul(
                    pvs[0][0:DK + 1, qoff:],
                    vaug_sb[:, kb, ds(vb, DK + 1)],
                    pt[:, qoff:512],
                    start=(kb == 0),
                    stop=(kb == nkb - 1),
                )
                nc.tensor.matmul(
                    pvs[1][:, qoff:],
                    vaug_sb[:, kb, ds(vb + DK, P)],
                    pt[:, 512 + qoff : 1024],
                    start=(kb == 0),
                    stop=(kb == nkb - 1),
                )

            # software-pipelined depth 2: scores/exp run one key block ahead
            # of PV so the PE never waits a full exp latency at block starts.
            # Own-chunk block j's compacted key j has pos >= 512c+j, so its
            # first 128j query columns are provably masked — trim them.
            pend = None
            for kb in range(nkb):
                dj = kb - 3 * qc               # >=0: own-chunk ragged block
                qoff = max(0, dj) * P
                w = 512 - qoff
                ps = psA.tile([P, 1024], F32, name=f"ps_a{qc}_{m}_{kb}", tag="ps")
                for hh in range(2):
                    r0 = hh * DK
                    nc.tensor.matmul(
                        ps[:, hh * 512 + qoff : (hh + 1) * 512],
                        kt_sb[r0 : r0 + DK, m, ds(kb * P, P)],
                        qt_sb[r0 : r0 + DK, m, ds(qc * 512 + qoff, w)],
                        start=True,
                        stop=(dj < 0),
                    )
                    if dj >= 0:
                        # += host mask (0 / -8e9) via eye-stationary matmul
                        nc.tensor.matmul(
                            ps[:, hh * 512 + qoff : (hh + 1) * 512],
                            eye_sb[:],
                            masks_sb[:, 3 * qc + dj, qoff:],
                            start=False,
                            stop=True,
                        )
                pt = pt_pool.tile([P, 1024], BF, name=f"pt{qc}_{m}_{kb}", tag="pt")
                ps3 = ps[:].rearrange("p (h q) -> p h q", h=2)[:, :, qoff:]
                pt3 = pt[:].rearrange("p (h q) -> p h q", h=2)[:, :, qoff:]
                nc.scalar.activation(pt3, ps3, AF.Exp, scale=0.125)
                if pend is not None:
                    emit_pv(*pend)
                    if fill and kb >= fill_start:
                        fill.pop(0)()
                pend = (kb, pt, qoff)
            emit_pv(*pend)
            # craws: even [0:65] = ctx_e+den_e; odd row 32 = den_o,
            # rows 64:128 = ctx_o (all quadrant-aligned accesses). crawA
            # rides the Act engine (idle at pair boundaries), crawB the DVE.
            crawA = rrp.tile([DK + 1, 512], F32, name=f"cA{qc}_{m}", tag="cA", bufs=3)
            nc.scalar.activation(crawA[:], pvs[0][0 : DK + 1, :], AF.Copy)
            crawB = rrp.tile([P, 512], F32, name=f"cB{qc}_{m}", tag="cB", bufs=3)
            nc.vector.tensor_copy(crawB[32:33, :], pvs[1][32:33, :])
            nc.vector.tensor_copy(crawB[DK:P, :], pvs[1][DK:P, :])
            for f in fill:
                f()
            return crawA, crawB

        def norm_collect(qc, m, craws):
            """Gather the pair's two denominators, take reciprocals (bf16),
            and shift the odd one to partition 0 for the PE broadcast."""
            crawA, crawB = craws
            den2 = rrp.tile([2, 512], F32, name=f"den2_{qc}_{m}", tag="den2", bufs=2)
            nc.gpsimd.dma_start(den2[0:1, :], crawA[DK : DK + 1, :])
            nc.gpsimd.dma_start(den2[1:2, :], crawB[32:33, :])
            rcp2 = rrp.tile([2, 512], F32, name=f"rcp2_{qc}_{m}", tag="rcp2", bufs=2)
            nc.vector.reciprocal_approx_fast(rcp2[:], den2[:])
            rcpb = rrp.tile([2, 512], BF, name=f"rcpb{qc}_{m}", tag="rcpb", bufs=2)
            nc.vector.tensor_copy(rcpb[:], rcp2[:])
            rcpb2 = rrp.tile([1, 512], BF, name=f"rcpc{qc}_{m}", tag="rcpc", bufs=2)
            nc.gpsimd.dma_start(rcpb2[:], rcpb[1:2, :])
            return rcpb, rcpb2

        def norm_mul(qc, m, craws, state):
            """Broadcast 1/den to the pair's 128 partitions with two K=1
            matmuls and normalize into the fp32r ctx pair tile."""
            crawA, crawB = craws
            rcpb, rcpb2 = state
            dst = ctx_sets[qc % 2][m]
            rb_ps = psY.tile([P, 512], F32, name=f"rbp{qc}_{m}", tag="yp")
            nc.tensor.matmul(
                rb_ps[0:DK, :], ones_bf[:], rcpb[0:1, :], start=True, stop=True
            )
            nc.tensor.matmul(
                rb_ps[DK:P, :], ones_bf[:], rcpb2[:], start=True, stop=True
            )
            nc.vector.tensor_mul(dst[0:DK, :], crawA[0:DK, :], rb_ps[0:DK, :])
            nc.vector.tensor_mul(dst[DK:P, :], crawB[DK:P, :], rb_ps[DK:P, :])

        def outproj_quanta(qc):
            """Output projection for chunk qc as single-matmul emitters."""
            held = {}
            quanta = []
            for si in range(4):
                s = qc * 4 + si
                for nch in range(2):
                    for hp in range(2):
                        def q(si=si, s=s, nch=nch, hp=hp):
                            if nch == 0 and hp == 0:
                                held["yt"] = ysb.tile(
                                    [P, 1024], F32, name=f"yt{s}", tag="yt"
                                )
                            if hp == 0:
                                held["yp"] = psY.tile(
                                    [P, 512], F32, name=f"yp{s}_{nch}", tag="yp"
                                )
                            yp = held["yp"]
                            nc.tensor.matmul(
                                yp[:],
                                ctx_sets[qc % 2][hp][:, ts(si, P)],
                                wo_sb[:, hp, ds(nch * 512, 512)],
                                start=(hp == 0),
                                stop=(hp == 1),
                            )
                            if hp == 1:
                                yt = held["yt"]
                                nc.vector.tensor_copy(yt[:, ts(nch, 512)], yp[:])
                                # stream each half out as soon as it lands —
                                # shortens the final-tile tail
                                nc.sync.dma_start(
                                    y_r[s][:, ts(nch, 512)], yt[:, ts(nch, 512)]
                                )
                        quanta.append(q)
            return quanta

        # ---------------- interleaved schedule ----------------
        # Emission order IS the per-engine execution order. qc order 1,2,3,0;
        # proj chunks 2,3 ride inside qc=1's pairs; the previous chunk's
        # output projection is dripped into the attention loop one matmul
        # per key block; norm muls are emitted after the interleaved work so
        # the reciprocal-broadcast latency is covered.
        proj_q(0)
        proj_kv(0)
        proj_q(1)
        proj_kv(1)
        prev = None
        for qc in QC_ORDER:
            last = qc == QC_ORDER[-1]
            quanta = outproj_quanta(prev) if prev is not None else []
            for m in range(2):
                if last:
                    # tail chunk: its key-block loops are too short to drip
                    # into — emit the previous chunk's outproj as a block
                    # right where it covers this pair's reciprocal chain
                    craws = scores_pair(qc, m, fill=())
                    state = norm_collect(qc, m, craws)
                    for q in quanta[8 * m : 8 * (m + 1)]:
                        q()
                    norm_mul(qc, m, craws, state)
                else:
                    craws = scores_pair(qc, m, fill=quanta[8 * m : 8 * (m + 1)])
                    state = norm_collect(qc, m, craws)
                    if qc == 1:
                        if m == 0:
                            proj_kv(2)
                            proj_q(2)
                        else:
                            proj_q(3)
                    norm_mul(qc, m, craws, state)
            prev = qc
        for q in outproj_quanta(QC_ORDER[-1]):
            q()


# ---------------- host side ----------------

def make_in_maps(x, padding_mask, Wq, bq, Wk, Wv, Wo):
    """Build the 8 per-core input dicts from full inputs. Keys are compacted
    per 512-token chunk: each chunk's nonpadded tokens go into CW=384 slots
    (zero-filled dummies beyond), past-chunk blocks are then exactly causal
    and only the diagonal chunk needs ragged masks."""
    bf16 = ml_dtypes.bfloat16
    x = np.asarray(x, dtype=np.float32)
    pad = np.asarray(padding_mask)
    eye = np.eye(P, dtype=np.float32).astype(bf16)
    B = x.shape[0]
    xtk_b, pad01_b, masks_b = [], [], []
    for b in range(B):
        xt = x[b].T  # [D, S] fp32
        xtk = np.zeros((D, SK), dtype=np.float32)
        p01 = np.zeros(SK, dtype=np.float32)
        masks = np.zeros((P, 12, 512), dtype=np.float32)
        for c in range(NQC):
            idx = np.flatnonzero(pad[b, 512 * c : 512 * (c + 1)] != 0) + 512 * c
            nk = len(idx)
            assert nk <= CW, f"chunk {c} has {nk} > {CW} nonpadded keys"
            xtk[:, CW * c : CW * c + nk] = xt[:, idx]
            p01[CW * c : CW * c + nk] = 1.0
            # diagonal masks: key j (orig pos p_j) allowed for query q iff
            # p_j <= q; dummies always masked
            pos = np.full(CW, S + 1, dtype=np.int64)
            pos[:nk] = idx
            qpos = 512 * c + np.arange(512)
            mc = np.where(pos[:, None] <= qpos[None, :], 0.0, np.float32(NEG))
            masks[:, 3 * c : 3 * c + 3, :] = (
                mc.reshape(3, P, 512).transpose(1, 0, 2)
            )
        xtk_b.append(np.ascontiguousarray(xtk).astype(bf16))
        pad01_b.append(np.ascontiguousarray(p01.reshape(SKT, P).T))
        masks_b.append(
            np.ascontiguousarray(masks.reshape(P, 12 * 512)).astype(bf16)
        )
    xt_b = [np.ascontiguousarray(x[b].T).astype(bf16) for b in range(B)]
    in_maps = []
    for c in range(N_CORES):
        b, g = divmod(c, 4)
        R = slice(g * 256, g * 256 + 256)
        in_maps.append(
            {
                "xt": xt_b[b],
                "xtk": xtk_b[b],
                "wq": np.ascontiguousarray(np.asarray(Wq, np.float32)[R, :].T).astype(bf16),
                "wk": np.ascontiguousarray(np.asarray(Wk, np.float32)[R, :].T).astype(bf16),
                "wv": np.ascontiguousarray(np.asarray(Wv, np.float32)[R, :].T).astype(bf16),
                "wo": np.ascontiguousarray(np.asarray(Wo, np.float32)[:, R].T),
                "bq": np.ascontiguousarray(
                    np.asarray(bq, np.float32)[R].reshape(2, P).T
                ),
                "pad01": pad01_b[b],
                "masks": masks_b[b],
                "eye": eye,
            }
        )
    return in_maps


def postprocess(partials, x, padding_mask, Wv, bv, Wo, bo):
    """Sum per-core partials, add folded bias, fix fully-masked rows."""
    x = np.asarray(x, np.float32)
    pad = np.asarray(padding_mask)
    Wv = np.asarray(Wv, np.float32)
    bv = np.asarray(bv, np.float32)
    Wo = np.asarray(Wo, np.float32)
    bo = np.asarray(bo, np.float32)
    B = x.shape[0]
    y = np.zeros((B, S, D), dtype=np.float32)
    for c in range(N_CORES):
        y[c // 4] += partials[c]
    y += (Wo @ bv + bo)[None, None, :]
    # fully-masked rows (reference: uniform attention over all keys)
    for b in range(B):
        nz = np.flatnonzero(pad[b] != 0)
        q0 = int(nz[0]) if len(nz) else S
        if q0 > 0:
            ctx_u = x[b].mean(axis=0) @ Wv.T + bv
            y[b, :q0, :] = ctx_u @ Wo.T + bo
    return y


_NC_CACHE = {}


def _get_program():
    if "nc" not in _NC_CACHE:
        _NC_CACHE["nc"] = build_program()
    return _NC_CACHE["nc"]


def kernel(
    x, padding_mask, Wq, bq, Wk, bk, Wv, bv, Wo, bo
):
    from concourse.bass_utils import run_bass_kernel_spmd

    nc = _get_program()
    in_maps = make_in_maps(x, padding_mask, Wq, bq, Wk, Wv, Wo)
    res = run_bass_kernel_spmd(nc, in_maps, core_ids=list(range(N_CORES)))
    partials = [res.results[c]["y"] for c in range(N_CORES)]
    return postprocess(partials, x, padding_mask, Wv, bv, Wo, bo)


# revision 49
# speedup vs baseline: 1.1100x; 1.1100x over previous
"""Self-contained Trainium2 Bass kernel for causal multi-head attention.

Problem: B=2, S=2048, D=1024, H=16 heads (dk=64), fp32, causal + padding mask.
Sharding across 8 NeuronCores: core c -> batch c//4, head-group c%4 (4 heads).

Device-side design:
  - x, Wq, Wk, Wv cast to bf16 on host: input DMA halves (12MB -> 6MB) and
    every matmul streams 1 col/cycle (fp32r at K=64 ran at 2 cyc/col).
  - qT/kT stored transposed [dk, S] in bf16; scores computed transposed
    S_T[k, q] so no transposes are needed anywhere; scores/exp run one key
    block ahead of PV (software pipeline depth 2).
  - No max-subtraction in softmax (scores are O(+-10); exp cannot overflow).
  - vaug per key block, per head pair (192 cols):
      [v_even(64) | den_e | junk(31) | den_o | junk(31) | v_odd(64)]
    Even PV stationary = cols 0:65 -> psum rows 0:64 ctx_e, row 64 den_e.
    Odd PV stationary = cols 64:192 -> psum row 32 den_o (quadrant-aligned
    for the DVE copy), rows 64:128 ctx_o. This lands the odd head's context
    at partitions 64:128 for free, so the output projection runs with K=128
    fully utilized (2 heads per accumulation step).
  - Padding: v rows and den cols zeroed for padded keys, so garbage exp
    values at padded keys multiply zeros everywhere.
  - Causal: tri (-8e9 lower triangle, bf16) accumulated onto the diagonal
    score blocks by the PE itself (eye-stationary matmul) — keeps the DVE
    out of the scores critical path.
  - Q/K psum->SBUF copies and the crawA context copy run on the Act engine
    (Identity-with-bias / Copy share Exp's act table, so no table reloads);
    V scaling stays on DVE.
  - The attention inner loop is exp-paced (Act engine); output-projection
    matmuls are sliced into single-matmul quanta and dripped one-per-key-
    block into the next chunk's attention loop to absorb the PE idle.
  - 1/denominator broadcast to the pair's 128 partitions via two K=1 bf16
    PE matmuls (the broadcasts ride idle PE slots; no DRAM bounce), then
    multiplied into fp32r ctx pair tiles.
  - Chunks processed in order 1,2,3,0 so the tail chunk is the smallest;
    projections for chunks 2,3 ride inside qc=1's pairs.
Fully-masked rows (all keys up to q padded) produce garbage on device and
are overwritten on host with the uniform-attention reference value.
"""

import numpy as np
from contextlib import ExitStack

import ml_dtypes

import concourse.bass as bass
import concourse.bacc as bacc
import concourse.tile as tile
import concourse.mybir as mybir
from concourse.bass import ds, ts

F32 = mybir.dt.float32
FR = mybir.dt.float32r
BF = mybir.dt.bfloat16
AF = mybir.ActivationFunctionType

P = 128
S = 2048
D = 1024
HL = 4          # heads per core
DK = 64
KT = D // P     # 8 k-tiles over the model dim
ST = S // P     # 16 seq tiles
NQC = 4         # 512-wide query chunks
NEG = -8.0e9    # pre-scale mask value; *0.125 = -1e9 -> exp underflows to 0
N_CORES = 8
N_HEAD = 16

PW = 192                     # cols per head pair in vaug (aligned den rows)
QC_ORDER = (1, 2, 3, 0)      # tail chunk is the smallest


def build_program(num_devices=N_CORES):
    nc = bacc.Bacc(
        "TRN2",
        target_bir_lowering=False,
        debug=False,
        enable_asserts=True,
        num_devices=num_devices,
    )
    ins = {
        "xt": nc.dram_tensor("xt", [D, S], BF, kind="ExternalInput").ap(),
        "wq": nc.dram_tensor("wq", [D, 2 * P], BF, kind="ExternalInput").ap(),
        "wk": nc.dram_tensor("wk", [D, 2 * P], BF, kind="ExternalInput").ap(),
        "wv": nc.dram_tensor("wv", [D, 2 * P], BF, kind="ExternalInput").ap(),
        "wo": nc.dram_tensor("wo", [2 * P, D], FR, kind="ExternalInput").ap(),
        "bq": nc.dram_tensor("bq", [P, 2], F32, kind="ExternalInput").ap(),
        "pad01": nc.dram_tensor("pad01", [P, ST], F32, kind="ExternalInput").ap(),
        "tri": nc.dram_tensor("tri", [P, P], BF, kind="ExternalInput").ap(),
        "eye": nc.dram_tensor("eye", [P, P], BF, kind="ExternalInput").ap(),
    }
    y = nc.dram_tensor("y", [S, D], F32, kind="ExternalOutput").ap()

    with tile.TileContext(nc) as tc:
        _body(tc, y, ins)

    nc.compile()
    return nc


def _body(tc, y, ins):
    nc = tc.nc

    with ExitStack() as ctx:
        const = ctx.enter_context(tc.tile_pool(name="const", bufs=1))
        pt_pool = ctx.enter_context(tc.tile_pool(name="pt", bufs=4))
        rrp = ctx.enter_context(tc.tile_pool(name="rr", bufs=2))
        ysb = ctx.enter_context(tc.tile_pool(name="ysb", bufs=2))
        psA = ctx.enter_context(tc.tile_pool(name="psA", bufs=2, space="PSUM"))
        psB = ctx.enter_context(tc.tile_pool(name="psB", bufs=2, space="PSUM"))
        psY = ctx.enter_context(tc.tile_pool(name="psY", bufs=2, space="PSUM"))

        # warmup operands first so the PE ramps while inputs stream
        ones_sb = const.tile([1, 512], BF)
        nc.vector.memset(ones_sb[:], 1.0)
        ones_bf = const.tile([1, DK], BF)
        nc.vector.memset(ones_bf[:], 1.0)

        # ---------------- input DMAs ----------------
        # all weights first (small, spread over the queues), then xt chunks
        xt_sb = const.tile([P, KT, S], BF)
        wq_sb = const.tile([P, KT, 2 * P], BF)
        wk_sb = const.tile([P, KT, 2 * P], BF)
        wv_sb = const.tile([P, KT, 2 * P], BF)
        xt_r = ins["xt"].rearrange("(k p) s -> k p s", p=P)
        w_rs = {n: ins[n].rearrange("(k p) n -> k p n", p=P) for n in ("wq", "wk", "wv")}

        bq_sb = const.tile([P, 2], F32)
        nc.sync.dma_start(bq_sb[:], ins["bq"])
        pad01_sb = const.tile([P, ST], F32)
        nc.sync.dma_start(pad01_sb[:], ins["pad01"])
        tri_sb = const.tile([P, P], BF)
        nc.sync.dma_start(tri_sb[:], ins["tri"])
        eye_sb = const.tile([P, P], BF)
        nc.sync.dma_start(eye_sb[:], ins["eye"])

        # xt chunk 0 first — each tile gets its own queue with nothing ahead
        # of it (the PE warmup covers the weight-DMA latency); then wq/wk,
        # then wv with chunk 1, then chunks 2+3
        for k in range(KT):
            nc.sync.dma_start(xt_sb[:, k, 0:512], xt_r[k][:, 0:512])
        for k in range(KT):
            nc.sync.dma_start(wq_sb[:, k], w_rs["wq"][k])
            nc.sync.dma_start(wk_sb[:, k], w_rs["wk"][k])
        for k in range(KT):
            nc.sync.dma_start(wv_sb[:, k], w_rs["wv"][k])
            nc.sync.dma_start(xt_sb[:, k, 512:1024], xt_r[k][:, 512:1024])
        for k in range(KT):
            nc.sync.dma_start(xt_sb[:, k, 1024:2048], xt_r[k][:, 1024:2048])

        # wo rows for a head pair are contiguous 128 rows -> K=128 all real
        wo_sb = const.tile([P, 2, D], FR)
        wo_r = ins["wo"].rearrange("(hp p) n -> hp p n", p=P)
        for hp in range(2):
            nc.sync.dma_start(wo_sb[:, hp], wo_r[hp])

        qt_sb = const.tile([P, 2, S], BF)
        kt_sb = const.tile([P, 2, S], BF)
        vaug_sb = const.tile([P, ST, 2 * PW], BF)
        for m in range(2):
            nc.vector.memset(vaug_sb[:, :, m * PW + DK + 1 : m * PW + 2 * DK], 0.0)

        # normalized context pairs [even dims 0:64 | odd dims 64:128], fp32r
        ctx_sets = [
            [
                const.tile([P, 512], FR, name=f"ctxsb{st}_{hp}", tag=f"ctxsb{st}_{hp}")
                for hp in range(2)
            ]
            for st in range(2)
        ]

        # PE warmup while the input DMAs stream (HAM un-throttle needs
        # ~3.4us of sustained matmul activity; these are dep-free)
        warm_ps = psY.tile([P, 512], F32, name="warm", tag="yp")

        def warm(n):
            """Dep-free PE filler: keeps the HAM clock ramped across input-
            DMA catch-up waits (a throttled PE runs matmuls at 1.2GHz)."""
            for _ in range(n):
                nc.tensor.matmul(
                    warm_ps[:], ones_sb[:, 0:P], ones_sb[:], start=True, stop=True
                )

        warm(24)

        # ---------------- projections for one 512-token chunk ----------------
        def proj_chunk(n):
            for tgt, w_sb, bias in ((qt_sb, wq_sb, bq_sb), (kt_sb, wk_sb, None)):
                ps = psA.tile([P, 1024], F32, name=f"ps_p{n}", tag="ps")
                for m in range(2):
                    for k in range(KT):
                        nc.tensor.matmul(
                            ps[:, ts(m, 512)],
                            w_sb[:, k, ts(m, P)],
                            xt_sb[:, k, ds(n * 512, 512)],
                            start=(k == 0),
                            stop=(k == KT - 1),
                        )

                # psum->SBUF on the Act engine (same act table as Exp)
                for m in range(2):
                    out_ap = tgt[:, m, ds(n * 512, 512)]
                    if bias is not None:
                        nc.scalar.activation(
                            out_ap, ps[:, ts(m, 512)], AF.Identity,
                            bias=bias[:, m : m + 1],
                        )
                    else:
                        nc.scalar.activation(out_ap, ps[:, ts(m, 512)], AF.Copy)
            ps = psA.tile([P, 1024], F32, name=f"ps_v{n}", tag="ps")
            for si in range(4):
                s = n * 4 + si
                for k in range(KT):
                    nc.tensor.matmul(
                        ps[:, ts(si, 256)],
                        xt_sb[:, k, ts(s, P)],
                        wv_sb[:, k, :],
                        start=(k == 0),
                        stop=(k == KT - 1),
                    )

            for si in range(4):
                s = n * 4 + si
                for h in range(HL):
                    m, odd = divmod(h, 2)
                    dst = m * PW + (2 * DK if odd else 0)
                    nc.vector.tensor_scalar_mul(
                        vaug_sb[:, s, ds(dst, DK)],
                        ps[:, ds(si * 256 + h * DK, DK)],
                        pad01_sb[:, s : s + 1],
                    )
                # den_e at col 64, den_o at col 96 of each pair block
                den_ap = vaug_sb[:, s, 0 : 2 * PW].rearrange(
                    "p (g c) -> p g c", c=32
                )
                for m in range(2):
                    nc.vector.tensor_copy(
                        den_ap[:, 6 * m + 2 : 6 * m + 4, 0:1],
                        pad01_sb[:, s : s + 1].to_broadcast([P, 2, 1]),
                    )

        # ---------------- attention for one 512-query chunk ----------------
        y_r = y.rearrange("(t p) n -> t p n", p=P)

        def emit_scores(qc, m, kb):
            """Scores (+causal tri) and exp for one key block of a pair.
            Returns the pending-PV descriptor."""
            dd = kb - 4 * qc
            qoff = max(0, dd) * P
            w = 512 - qoff
            ps = psA.tile([P, 1024], F32, name=f"ps_a{qc}_{m}_{kb}", tag="ps")
            for hh in range(2):
                r0 = hh * DK
                nc.tensor.matmul(
                    ps[:, hh * 512 + qoff : (hh + 1) * 512],
                    kt_sb[r0 : r0 + DK, m, ds(kb * P, P)],
                    qt_sb[r0 : r0 + DK, m, ds(qc * 512 + qoff, w)],
                    start=True,
                    stop=(dd < 0),
                )
                if dd >= 0:
                    # causal mask: += tri on the diagonal 128 block,
                    # done by the PE (eye stationary, tri moving)
                    nc.tensor.matmul(
                        ps[:, hh * 512 + qoff : hh * 512 + qoff + P],
                        eye_sb[:],
                        tri_sb[:],
                        start=False,
                        stop=True,
                    )
            pt = pt_pool.tile([P, 1024], BF, name=f"pt{qc}_{m}_{kb}", tag="pt")
            ps3 = ps[:].rearrange("p (h q) -> p h q", h=2)[:, :, qoff:]
            pt3 = pt[:].rearrange("p (h q) -> p h q", h=2)[:, :, qoff:]
            nc.scalar.activation(pt3, ps3, AF.Exp, scale=0.125)
            return (kb, pt, qoff)

        def scores_pair(qc, m, fill=(), prol=None, next_fn=None):
            """QK^T, exp, PV for head pair (2m, 2m+1). `fill` items drip one
            per key block to soak up the exp-paced PE idle. `prol` is this
            pair's first block, already emitted inside the previous pair;
            `next_fn` emits the NEXT pair's first block just before our last
            PV, giving the Act engine a head start across the boundary."""
            fill = list(fill)
            nkb = 4 * qc + 4
            fill_start = 2 if nkb <= 4 else 4
            vb = m * PW
            pvs = [
                psB.tile([P, 512], F32, name=f"ctx{qc}_{m}_{i}", tag="ctx")
                for i in range(2)
            ]

            def emit_pv(kb, pt, qoff):
                nc.tensor.matmul(
                    pvs[0][0:DK + 1, qoff:],
                    vaug_sb[:, kb, ds(vb, DK + 1)],
                    pt[:, qoff:512],
                    start=(kb == 0),
                    stop=(kb == nkb - 1),
                )
                nc.tensor.matmul(
                    pvs[1][:, qoff:],
                    vaug_sb[:, kb, ds(vb + DK, P)],
                    pt[:, 512 + qoff : 1024],
                    start=(kb == 0),
                    stop=(kb == nkb - 1),
                )

            # software-pipelined depth 2: scores/exp run one key block ahead
            # of PV so the PE never waits a full exp latency at block starts
            pend = prol
            for kb in range(1 if prol is not None else 0, nkb):
                cur = emit_scores(qc, m, kb)
                if pend is not None:
                    emit_pv(*pend)
                    if fill and kb >= fill_start:
                        fill.pop(0)()
                pend = cur
            nxt = next_fn() if next_fn is not None else None
            emit_pv(*pend)
            # craws: even [0:65] = ctx_e+den_e; odd row 32 = den_o,
            # rows 64:128 = ctx_o (all quadrant-aligned accesses). crawA
            # rides the Act engine (idle at pair boundaries), crawB the DVE.
            crawA = rrp.tile([DK + 1, 512], F32, name=f"cA{qc}_{m}", tag="cA", bufs=3)
            nc.scalar.activation(crawA[:], pvs[0][0 : DK + 1, :], AF.Copy)
            crawB = rrp.tile([P, 512], F32, name=f"cB{qc}_{m}", tag="cB", bufs=3)
            nc.vector.tensor_copy(crawB[32:33, :], pvs[1][32:33, :])
            nc.vector.tensor_copy(crawB[DK:P, :], pvs[1][DK:P, :])
            for f in fill:
                f()
            return crawA, crawB, nxt

        def norm_collect(qc, m, craws):
            """Gather the pair's two denominators, take reciprocals (bf16),
            and shift the odd one to partition 0 for the PE broadcast."""
            crawA, crawB = craws
            den2 = rrp.tile([2, 512], F32, name=f"den2_{qc}_{m}", tag="den2", bufs=2)
            nc.gpsimd.dma_start(den2[0:1, :], crawA[DK : DK + 1, :])
            nc.gpsimd.dma_start(den2[1:2, :], crawB[32:33, :])
            rcp2 = rrp.tile([2, 512], F32, name=f"rcp2_{qc}_{m}", tag="rcp2", bufs=2)
            nc.vector.reciprocal_approx_fast(rcp2[:], den2[:])
            rcpb = rrp.tile([2, 512], BF, name=f"rcpb{qc}_{m}", tag="rcpb", bufs=2)
            nc.vector.tensor_copy(rcpb[:], rcp2[:])
            rcpb2 = rrp.tile([1, 512], BF, name=f"rcpc{qc}_{m}", tag="rcpc", bufs=2)
            nc.gpsimd.dma_start(rcpb2[:], rcpb[1:2, :])
            return rcpb, rcpb2

        def norm_mul(qc, m, craws, state):
            """Broadcast 1/den to the pair's 128 partitions with two K=1
            matmuls and normalize into the fp32r ctx pair tile."""
            crawA, crawB = craws
            rcpb, rcpb2 = state
            dst = ctx_sets[qc % 2][m]
            rb_ps = psY.tile([P, 512], F32, name=f"rbp{qc}_{m}", tag="yp")
            nc.tensor.matmul(
                rb_ps[0:DK, :], ones_bf[:], rcpb[0:1, :], start=True, stop=True
            )
            nc.tensor.matmul(
                rb_ps[DK:P, :], ones_bf[:], rcpb2[:], start=True, stop=True
            )
            nc.vector.tensor_mul(dst[0:DK, :], crawA[0:DK, :], rb_ps[0:DK, :])
            nc.vector.tensor_mul(dst[DK:P, :], crawB[DK:P, :], rb_ps[DK:P, :])

        def outproj_quanta(qc):
            """Output projection for chunk qc as single-matmul emitters."""
            held = {}
            quanta = []
            for si in range(4):
                s = qc * 4 + si
                for nch in range(2):
                    for hp in range(2):
                        def q(si=si, s=s, nch=nch, hp=hp):
                            if nch == 0 and hp == 0:
                                held["yt"] = ysb.tile(
                                    [P, 1024], F32, name=f"yt{s}", tag="yt"
                                )
                            if hp == 0:
                                held["yp"] = psY.tile(
                                    [P, 512], F32, name=f"yp{s}_{nch}", tag="yp"
                                )
                            yp = held["yp"]
                            nc.tensor.matmul(
                                yp[:],
                                ctx_sets[qc % 2][hp][:, ts(si, P)],
                                wo_sb[:, hp, ds(nch * 512, 512)],
                                start=(hp == 0),
                                stop=(hp == 1),
                            )
                            if hp == 1:
                                yt = held["yt"]
                                nc.vector.tensor_copy(yt[:, ts(nch, 512)], yp[:])
                                # stream each half out as soon as it lands —
                                # shortens the final-tile tail
                                nc.sync.dma_start(
                                    y_r[s][:, ts(nch, 512)], yt[:, ts(nch, 512)]
                                )
                        quanta.append(q)
            return quanta

        # ---------------- interleaved schedule ----------------
        # Emission order IS the per-engine execution order. qc order 1,2,3,0;
        # proj chunks 2,3 ride inside qc=1's pairs; the previous chunk's
        # output projection is dripped into the attention loop one matmul
        # per key block; norm muls are emitted after the interleaved work so
        # the reciprocal-broadcast latency is covered.
        proj_chunk(0)
        proj_chunk(1)
        seq = [(qc, m) for qc in QC_ORDER for m in range(2)]
        prol = None
        prev = None
        quanta = []
        for i, (qc, m) in enumerate(seq):
            last_chunk = qc == QC_ORDER[-1]
            if m == 0:
                quanta = outproj_quanta(prev) if prev is not None else []
            nxt = seq[i + 1] if i + 1 < len(seq) else None
            next_fn = (
                (lambda q=nxt: emit_scores(q[0], q[1], 0)) if nxt else None
            )
            fillseq = () if last_chunk else quanta[8 * m : 8 * (m + 1)]
            craws0, craws1, prol = scores_pair(
                qc, m, fill=fillseq, prol=prol, next_fn=next_fn
            )
            craws = (craws0, craws1)
            state = norm_collect(qc, m, craws)
            if last_chunk:
                # tail chunk: its key-block loops are too short to drip
                # into — emit the previous chunk's outproj as a block
                # right where it covers this pair's reciprocal chain
                for q in quanta[8 * m : 8 * (m + 1)]:
                    q()
            elif qc == 1:
                proj_chunk(2 + m)
            norm_mul(qc, m, craws, state)
            if m == 1:
                prev = qc
        for q in outproj_quanta(QC_ORDER[-1]):
            q()


# ---------------- host side ----------------

def make_in_maps(x, padding_mask, Wq, bq, Wk, Wv, Wo):
    """Build the 8 per-core input dicts from full inputs."""
    bf16 = ml_dtypes.bfloat16
    x = np.asarray(x, dtype=np.float32)
    pad = np.asarray(padding_mask)
    tri = np.where(
        np.arange(P)[:, None] > np.arange(P)[None, :], np.float32(NEG), np.float32(0)
    ).astype(bf16)
    eye = np.eye(P, dtype=np.float32).astype(bf16)
    xt_b = [np.ascontiguousarray(x[b].T).astype(bf16) for b in range(x.shape[0])]
    in_maps = []
    for c in range(N_CORES):
        b, g = divmod(c, 4)
        R = slice(g * 256, g * 256 + 256)
        pad01 = (pad[b] != 0).astype(np.float32).reshape(ST, P).T.copy()
        in_maps.append(
            {
                "xt": xt_b[b],
                "wq": np.ascontiguousarray(np.asarray(Wq, np.float32)[R, :].T).astype(bf16),
                "wk": np.ascontiguousarray(np.asarray(Wk, np.float32)[R, :].T).astype(bf16),
                "wv": np.ascontiguousarray(np.asarray(Wv, np.float32)[R, :].T).astype(bf16),
                "wo": np.ascontiguousarray(np.asarray(Wo, np.float32)[:, R].T),
                "bq": np.ascontiguousarray(
                    np.asarray(bq, np.float32)[R].reshape(2, P).T
                ),
                "pad01": pad01,
                "tri": tri,
                "eye": eye,
            }
        )
    return in_maps


def postprocess(partials, x, padding_mask, Wv, bv, Wo, bo):
    """Sum per-core partials, add folded bias, fix fully-masked rows."""
    x = np.asarray(x, np.float32)
    pad = np.asarray(padding_mask)
    Wv = np.asarray(Wv, np.float32)
    bv = np.asarray(bv, np.float32)
    Wo = np.asarray(Wo, np.float32)
    bo = np.asarray(bo, np.float32)
    B = x.shape[0]
    y = np.zeros((B, S, D), dtype=np.float32)
    for c in range(N_CORES):
        y[c // 4] += partials[c]
    y += (Wo @ bv + bo)[None, None, :]
    # fully-masked rows (reference: uniform attention over all keys)
    for b in range(B):
        nz = np.flatnonzero(pad[b] != 0)
        q0 = int(nz[0]) if len(nz) else S
        if q0 > 0:
            ctx_u = x[b].mean(axis=0) @ Wv.T + bv
            y[b, :q0, :] = ctx_u @ Wo.T + bo
    return y


_NC_CACHE = {}


def _get_program():
    if "nc" not in _NC_CACHE:
        _NC_CACHE["nc"] = build_program()
    return _NC_CACHE["nc"]


def kernel(
    x, padding_mask, Wq, bq, Wk, bk, Wv, bv, Wo, bo
):
    from concourse.bass_utils import run_bass_kernel_spmd

    nc = _get_program()
    in_maps = make_in_maps(x, padding_mask, Wq, bq, Wk, Wv, Wo)
    res = run_bass_kernel_spmd(nc, in_maps, core_ids=list(range(N_CORES)))
    partials = [res.results[c]["y"] for c in range(N_CORES)]
    return postprocess(partials, x, padding_mask, Wv, bv, Wo, bo)
